# revision 1
# baseline (speedup 1.0000x reference)
"""FLA GatedDeltaNet layer on 8 Trainium2 NeuronCores.

Sharding: data-parallel over batch (2 groups) x tensor-parallel over heads
(4 shards of 2 heads). Each core computes its batch element with its 2 heads
end-to-end (projections, short conv, gated delta rule recurrence, gated
RMSNorm, o_proj partial). Host sums the 4 o_proj partials per batch element.

Recurrence uses the chunked WY form (chunk C=128):
  S_t = exp(g_t) S_{t-1};  u_t = beta_t (v_t - k_t^T S_t);  S_t += k_t u_t^T
  per chunk: (I + N) U = beta (V - Lam K S0),  N[t,i] = b_t e^{c_t-c_i} k_t.k_i
  TmT = transposed (I+N)^{-1} via product-form doubling (N nilpotent),
  O = Lam Q S0 + (QK^T . decay) U,  S1 = gam S0 + ((gam/Lam) K)^T U.

Matmul dtypes: float32r (fast fp32 mode) for all N>=256 matmuls, bf16 for the
[C,C] gram/inverse matmuls, exact fp32 for the decay broadcast matmuls.
"""
import sys

if "/opt/trn_rl_repo" not in sys.path:
    sys.path.insert(0, "/opt/trn_rl_repo")

import numpy as np
import ml_dtypes

import concourse.bass as bass
import concourse.bacc as bacc
import concourse.mybir as mybir
import concourse.tile as tile
from concourse.bass_utils import run_bass_kernel_spmd

F32 = mybir.dt.float32
F32R = mybir.dt.float32r
BF16 = mybir.dt.bfloat16
AF = mybir.ActivationFunctionType
OP = mybir.AluOpType

B, T, D = 2, 1024, 1024
H, DK, DV, KC = 8, 128, 256, 4
HL = 2              # heads per core
C = 128             # chunk length
NCH = T // C        # 8 chunks
NEG = -1e30
EPS = 1e-5
QSCALE = float(DK) ** -0.5

_cache = {}


def build_kernel(debug=False):
    nc = bacc.Bacc(None, target_bir_lowering=False)

    xT = nc.dram_tensor("xT", [D, T], F32R, kind="ExternalInput")
    Wqk = nc.dram_tensor("Wqk", [D, 512], F32R, kind="ExternalInput")
    Wv = nc.dram_tensor("Wv", [D, 512], F32R, kind="ExternalInput")
    Wg = nc.dram_tensor("Wg", [D, 512], F32R, kind="ExternalInput")
    Wba = nc.dram_tensor("Wba", [D, 4], F32R, kind="ExternalInput")
    Wo = nc.dram_tensor("Wo", [HL * DV, D], F32R, kind="ExternalInput")
    convd = nc.dram_tensor("convd", [8, KC, 128, 128], F32R, kind="ExternalInput")
    adt = nc.dram_tensor("adt", [16, 2], F32, kind="ExternalInput")
    maskI = nc.dram_tensor("maskI", [128, 128], F32, kind="ExternalInput")
    maskS = nc.dram_tensor("maskS", [128, 128], F32, kind="ExternalInput")
    identf = nc.dram_tensor("identf", [128, 128], F32R, kind="ExternalInput")
    identg = nc.dram_tensor("identg", [16, 16], F32, kind="ExternalInput")
    identb = nc.dram_tensor("identb", [128, 128], BF16, kind="ExternalInput")
    outD = nc.dram_tensor("out", [T, D], F32, kind="ExternalOutput")
    dbg = {}
    if debug:
        for nm, shp in [("qk0", [128, T]), ("qk2", [128, T]), ("v0", [128, T]),
                        ("ba", [4, T]), ("crows", [16, 128]), ("lam", [16, 128]),
                        ("ed", [16, 128]), ("bet", [16, 128]),
                        ("TmT00", [128, 128]), ("AT00", [128, 128]),
                        ("Vb00", [128, 256]), ("Kp00", [128, 128]),
                        ("S0", [128, DV]), ("og", [128, NCH * 2 * DV])]:
            dbg[nm] = nc.dram_tensor("dbg_" + nm, shp, F32, kind="ExternalOutput")

    with tile.TileContext(nc, pool_alloc_mode="queue") as tc, \
         tc.tile_pool(name="res", bufs=1) as res:

        # ---------------- resident (small) loads ----------------
        Wo_s = res.tile([128, 4, D], F32R)
        Wba_s = res.tile([128, 8, 4], F32R)
        nc.sync.dma_start(out=Wba_s, in_=Wba.rearrange("(dt p) c -> p dt c", p=128))
        adt_s = res.tile([16, 2], F32)
        nc.sync.dma_start(out=adt_s, in_=adt[:, :])

        ones_row = res.tile([1, 128], F32)
        nc.vector.memset(ones_row, 1.0)
        zerf_col = res.tile([128, 4], F32)
        nc.vector.memset(zerf_col, 0.0)
        zeror_col = res.tile([128, 4], F32R)
        nc.vector.tensor_copy(zeror_col[:, :], zerf_col[:, :])
        onesb_col = res.tile([128, 1], BF16)
        nc.vector.memset(onesb_col, 1.0)
        eps6_col = res.tile([128, 1], F32)
        nc.vector.memset(eps6_col, 1e-6)
        epsn_col = res.tile([128, 1], F32)
        nc.vector.memset(epsn_col, EPS)

        # persistent per-block activation tiles
        qkT = [res.tile([128, T], F32R, tag=f"qkT{i}", name=f"qkT{i}") for i in range(4)]
        qkB_all = res.tile([128, 4, T], BF16)   # block order: k0 q0 k1 q1
        nsq_s = [res.tile([1, T], F32, tag=f"nsq{i}", name=f"nsq{i}")
                 for i in range(4)]
        vT = [res.tile([128, T], F32R, tag=f"vT{i}", name=f"vT{i}") for i in range(4)]
        ba_s = res.tile([4, T], F32)
        gvsnw = res.tile([128, NCH, 2 * DV], F32)     # silu(gv) * norm_w, row-major
        og_s = res.tile([128, NCH, 2 * DV], F32R)      # o (later gated), row-major
        S_s = [res.tile([128, DV], F32R, tag=f"S{hl}", name=f"S{hl}") for hl in range(HL)]
        ssq_all = res.tile([128, 16], F32)
        rstd_all = res.tile([128, 16], F32)

        # ======== projection scope (xT/Wg freed afterwards) ========
        with tc.tile_pool(name="xp", bufs=1) as xp, \
             tc.tile_pool(name="wstream", bufs=3) as wstream, \
             tc.tile_pool(name="cstream", bufs=2) as cstream, \
             tc.tile_pool(name="pre", bufs=2) as pre_pool, \
             tc.tile_pool(name="psJ", bufs=4, space="PSUM") as psJ, \
             tc.tile_pool(name="psn", bufs=2, space="PSUM") as psn:

            xT_s = xp.tile([128, 8, T], F32R)          # [p, dtile, t]
            for dt_i in range(8):
                nc.sync.dma_start(
                    out=xT_s[:, dt_i, :],
                    in_=xT.rearrange("(dt p) t -> p dt t", p=128)[:, dt_i, :])
            Wg_s = xp.tile([128, 8, 512], F32R)

            # ---- transposed projections: ba first, then q,k,v c-blocks ----
            for blk in [8, 2, 0, 4, 5, 3, 1, 6, 7]:
                nparts = 128 if blk < 8 else 4
                psums = [psJ.tile([nparts, 512], F32, tag="psJ", name=f"pj{blk}_{h}")
                         for h in range(2)]
                if blk < 8:
                    wsrc = Wqk if blk < 4 else Wv
                    cb = blk if blk < 4 else blk - 4
                    wt8 = wstream.tile([128, 8, 128], F32R, tag="w", name=f"w{blk}")
                    nc.sync.dma_start(
                        out=wt8,
                        in_=wsrc.rearrange("(dt p) c -> p dt c", p=128)[:, :, cb * 128:(cb + 1) * 128])
                for d in range(8):
                    wt = wt8[:, d, :] if blk < 8 else Wba_s[:, d, :]
                    for half in range(2):
                        nc.tensor.matmul(
                            psums[half][:, :], wt,
                            xT_s[:, d, half * 512:(half + 1) * 512],
                            start=(d == 0), stop=(d == 7))
                if blk < 8:
                    pret = pre_pool.tile([128, T + 3], F32R, tag="pre", name=f"pre{blk}")
                    nc.vector.tensor_copy(pret[:, 0:3], zeror_col[:, 0:3])
                    nc.vector.tensor_copy(pret[:, 3:3 + 512], psums[0][:, :])
                    nc.scalar.activation(pret[:, 3 + 512:3 + 1024], psums[1][:, :],
                                         AF.Copy)
                    cdt = cstream.tile([128, KC, 128], F32R, tag="cd", name=f"cd{blk}")
                    nc.sync.dma_start(
                        out=cdt, in_=convd.rearrange("b k p m -> p b k m")[:, blk, :, :])
                    dest = vT[blk - 4] if blk >= 4 else qkT[blk]
                    cps2 = [psJ.tile([128, 512], F32, tag="psJ", name=f"cv{blk}_{h}")
                            for h in range(2)]
                    for tap in range(KC):
                        for half in range(2):
                            nc.tensor.matmul(
                                cps2[half][:, :], cdt[:, tap, :],
                                pret[:, half * 512 + tap:half * 512 + tap + 512],
                                start=(tap == 0), stop=(tap == KC - 1))
                    for half in range(2):
                        nc.scalar.activation(dest[:, half * 512:(half + 1) * 512],
                                             cps2[half][:, :], AF.Silu)
                    if blk < 4:
                        # l2 norm stats only; normalization is folded into the
                        # decay scalars (exp-space) in the recurrence scope
                        sq = pre_pool.tile([128, T], BF16, tag="sq", name=f"sq{blk}")
                        nc.scalar.activation(sq[:, :], dest[:, :], AF.Square)
                        for q4 in range(4):
                            nsp = psn.tile([1, 256], F32, tag="psn", name=f"ns{blk}_{q4}")
                            nc.tensor.matmul(nsp[:, :], onesb_col[:, :],
                                             sq[:, q4 * 256:(q4 + 1) * 256],
                                             start=True, stop=True)
                            nc.vector.tensor_copy(
                                nsq_s[blk][:, q4 * 256:(q4 + 1) * 256], nsp[:, :])
                        bmap = {0: 1, 1: 3, 2: 0, 3: 2}
                        nc.gpsimd.tensor_copy(qkB_all[:, bmap[blk], :], dest[:, :])
                else:
                    for half in range(2):
                        nc.vector.tensor_copy(ba_s[:, half * 512:(half + 1) * 512],
                                              psums[half][:, :])

            # ---- gv projection (row-major) + silu * norm_w ----
            for dt_i in range(8):
                nc.sync.dma_start(
                    out=Wg_s[:, dt_i, :],
                    in_=Wg.rearrange("(dt p) c -> p dt c", p=128)[:, dt_i, :])
            for tt in range(NCH):
                gps = psJ.tile([128, 512], F32, tag="psJ", name=f"gv{tt}")
                for d in range(8):
                    nc.tensor.matmul(gps[:, :], xT_s[:, d, tt * 128:(tt + 1) * 128],
                                     Wg_s[:, d, :], start=(d == 0), stop=(d == 7))
                nc.scalar.activation(gvsnw[:, tt, :], gps[:, :], AF.Silu)

        # ======== recurrence scope (reuses xT/Wg space) ========
        with tc.tile_pool(name="rc", bufs=1) as rc, \
             tc.tile_pool(name="phA", bufs=2) as phA, \
             tc.tile_pool(name="invp", bufs=4) as invp, \
             tc.tile_pool(name="phB", bufs=3) as phB, \
             tc.tile_pool(name="outp", bufs=2) as outp, \
             tc.tile_pool(name="psA", bufs=5, space="PSUM") as psA, \
             tc.tile_pool(name="psB", bufs=3, space="PSUM") as psB:

            maskIS_s = rc.tile([128, 256], F32)
            nc.sync.dma_start(out=maskIS_s[:, 0:128], in_=maskI[:, :])
            nc.sync.dma_start(out=maskIS_s[:, 128:256], in_=maskS[:, :])
            identf_s = rc.tile([128, 128], F32R)
            identg_s = rc.tile([16, 16], F32)
            nc.sync.dma_start(out=identg_s, in_=identg[:, :])
            nc.sync.dma_start(out=identf_s, in_=identf[:, :])
            identb_s = rc.tile([128, 128], BF16)
            nc.sync.dma_start(out=identb_s, in_=identb[:, :])
            zeros16 = rc.tile([16, 128], F32)
            nc.vector.memset(zeros16, 0.0)

            # ---- per-chunk scalar streams (l2 norms folded into exp space) ----
            bagB = rc.tile([16, 128], F32)  # beta-proj rows (h-major chunks)
            bagA = rc.tile([16, 128], F32)  # a-proj rows
            for f in range(2):
                nc.sync.dma_start(
                    out=bagB[f * NCH:(f + 1) * NCH, :],
                    in_=ba_s[f:f + 1, :].rearrange("o (j t) -> o j t", j=NCH))
                nc.sync.dma_start(
                    out=bagA[f * NCH:(f + 1) * NCH, :],
                    in_=ba_s[2 + f:3 + f, :].rearrange("o (j t) -> o j t", j=NCH))
            bet_rows = rc.tile([16, 128], F32)
            nc.scalar.activation(bet_rows[:, :], bagB[:, :], AF.Sigmoid)
            sg_rows = rc.tile([16, 128], F32)
            nc.scalar.activation(sg_rows[:, :], bagA[:, :], AF.Sigmoid,
                                 scale=-1.0, bias=adt_s[:, 0:1])
            # g = exp(A_log) * ln(sigmoid(-(a+dt_bias)))  [= -expA * softplus]
            g_rows = rc.tile([16, 128], F32)
            nc.scalar.activation(g_rows[:, :], sg_rows[:, :], AF.Ln)
            nc.scalar.activation(g_rows[:, :], g_rows[:, :], AF.Copy,
                                 scale=adt_s[:, 1:2])
            c_rows = rc.tile([16, 128], F32)
            nc.vector.tensor_tensor_scan(c_rows[:, :], g_rows[:, :], zeros16[:, :],
                                         0.0, op0=OP.add, op1=OP.add)
            lnb_rows = rc.tile([16, 128], F32)
            nc.scalar.activation(lnb_rows[:, :], bet_rows[:, :], AF.Ln)
            cb_rows = rc.tile([16, 128], F32)
            nc.vector.tensor_tensor(out=cb_rows[:, :], in0=c_rows[:, :],
                                    in1=lnb_rows[:, :], op=OP.add)
            lam_rows = rc.tile([16, 128], F32)
            nc.scalar.activation(lam_rows[:, :], c_rows[:, :], AF.Exp)

            # half-log norms: ln|q|, ln|k| in chunk-row layout
            lnq4 = [rc.tile([1, T], F32, tag=f"lnq{i}", name=f"lnq{i}")
                    for i in range(4)]
            for i in range(4):
                nc.scalar.activation(lnq4[i][:, :], nsq_s[i][:, :], AF.Ln,
                                     bias=eps6_col[0:1, :])
            lnm_rows = rc.tile([16, 128], F32)   # q-norm logs
            lnn_rows = rc.tile([16, 128], F32)   # k-norm logs
            for f in range(2):
                nc.sync.dma_start(
                    out=lnm_rows[f * NCH:(f + 1) * NCH, :],
                    in_=lnq4[f][0:1, :].rearrange("o (j t) -> o j t", j=NCH))
                nc.sync.dma_start(
                    out=lnn_rows[f * NCH:(f + 1) * NCH, :],
                    in_=lnq4[2 + f][0:1, :].rearrange("o (j t) -> o j t", j=NCH))

            cqs_rows = rc.tile([16, 128], F32)   # c + ln(qscale)
            nc.vector.tensor_scalar_add(cqs_rows[:, :], c_rows[:, :],
                                        float(np.log(QSCALE)))
            cA_rows = rc.tile([16, 128], F32)    # c + ln(qscale) - 0.5 ln|q|^2
            nc.vector.scalar_tensor_tensor(
                out=cA_rows[:, :], in0=lnm_rows[:, :], scalar=-0.5,
                in1=cqs_rows[:, :], op0=OP.mult, op1=OP.add)
            cN_rows = rc.tile([16, 128], F32)    # c + ln(b) - 0.5 ln|k| (N cols)
            nc.vector.scalar_tensor_tensor(
                out=cN_rows[:, :], in0=lnn_rows[:, :], scalar=-0.5,
                in1=cb_rows[:, :], op0=OP.mult, op1=OP.add)
            ccn_rows = rc.tile([16, 128], F32)   # c + 0.5 ln|k|  (decay row scalar)
            nc.vector.scalar_tensor_tensor(
                out=ccn_rows[:, :], in0=lnn_rows[:, :], scalar=0.5,
                in1=c_rows[:, :], op0=OP.mult, op1=OP.add)
            cln_rows = rc.tile([16, 128], F32)   # c - 0.5 ln|k|
            nc.vector.scalar_tensor_tensor(
                out=cln_rows[:, :], in0=lnn_rows[:, :], scalar=-0.5,
                in1=c_rows[:, :], op0=OP.mult, op1=OP.add)

            lamq_rows = rc.tile([16, 128], F32)  # qscale * lam / |q|
            nc.scalar.activation(lamq_rows[:, :], cA_rows[:, :], AF.Exp)
            edn_rows = rc.tile([16, 128], F32)   # exp(c_last - c)/|k|
            nc.scalar.activation(edn_rows[:, :], ccn_rows[:, :], AF.Exp,
                                 scale=-1.0, bias=c_rows[:, 127:128])
            lamn_rows = rc.tile([16, 128], F32)  # lam/|k|
            nc.scalar.activation(lamn_rows[:, :], cln_rows[:, :], AF.Exp)
            nbl_rows = rc.tile([16, 128], F32)   # -beta * lam / |k|
            nc.vector.tensor_tensor(out=nbl_rows[:, :], in0=bet_rows[:, :],
                                    in1=lamn_rows[:, :], op=OP.mult)
            nc.scalar.activation(nbl_rows[:, :], nbl_rows[:, :], AF.Copy, scale=-1.0)

            # flatten decay-col rows to single-partition tiles for PE rhs
            ccb_flat = rc.tile([1, 2, 16 * 128], F32)
            nc.sync.dma_start(
                out=ccb_flat[0:1, 0, :].rearrange("p (r t) -> p r t", r=16),
                in_=cA_rows[:, :])
            nc.sync.dma_start(
                out=ccb_flat[0:1, 1, :].rearrange("p (r t) -> p r t", r=16),
                in_=cN_rows[:, :])

            cols = {}
            for nm, rt in [("ccn", ccn_rows), ("lam", lamq_rows), ("ed", edn_rows),
                           ("b", bet_rows), ("nbl", nbl_rows)]:
                ps = psA.tile([128, 16], F32, tag="psA", name=f"tc_{nm}")
                nc.tensor.transpose(ps[:, :], rt[:, :], identg_s[:, :])
                ct = rc.tile([128, 16], F32, tag=f"cols_{nm}", name=f"cols_{nm}")
                nc.vector.tensor_copy(ct[:, :], ps[:, :])
                cols[nm] = ct
            ccn_cols, lam_cols, ed_cols, b_cols, nbl_cols = (
                cols["ccn"], cols["lam"], cols["ed"], cols["b"], cols["nbl"])

            glast_ps = psA.tile([1, 16], F32, tag="psA", name="glast_ps")
            nc.tensor.transpose(glast_ps[:, :], lam_rows[:, 127:128],
                                identg_s[:, :])
            glast_row = rc.tile([1, 16], F32)
            nc.vector.tensor_copy(glast_row[:, :], glast_ps[:, :])
            gamb_ps = psA.tile([128, 16], F32, tag="psA", name="gamb_ps")
            nc.tensor.matmul(gamb_ps[:, :], ones_row[:, :], glast_row[:, :],
                             start=True, stop=True)
            gamb_s = rc.tile([128, 16], F32)
            nc.vector.tensor_copy(gamb_s[:, :], gamb_ps[:, :])

            # ---- phase A: chunk-parallel precompute (j-major) ----
            TmT_t, AT_t, Vb_t, Kp_t = {}, {}, {}, {}
            for j in range(NCH):
                for hl in range(HL):
                    rj = hl * NCH + j
                    sl = slice(j * 128, (j + 1) * 128)
                    kb = qkB_all[:, 2 * hl, :]
                    kn = qkT[2 + hl]

                    gp = psA.tile([128, 256], F32, tag="psA", name=f"gp{rj}")
                    nc.tensor.matmul(gp[:, :].rearrange("p (b t) -> p b t", b=2),
                                     kb[:, sl], qkB_all[:, 2 * hl:2 * hl + 2, sl],
                                     start=True, stop=True)

                    bcAN = psA.tile([128, 256], F32, tag="psA", name=f"bcAN{rj}")
                    nc.tensor.matmul(bcAN[:, :].rearrange("p (b t) -> p b t", b=2),
                                     ones_row[:, :],
                                     ccb_flat[:, :, rj * 128:(rj + 1) * 128],
                                     start=True, stop=True)
                    dAN = phA.tile([128, 256], F32, tag="dAN", bufs=4, name=f"dAN{rj}")
                    nc.vector.scalar_tensor_tensor(
                        out=dAN[:, :], in0=bcAN[:, :],
                        scalar=ccn_cols[:, rj:rj + 1],
                        in1=maskIS_s[:, :], op0=OP.subtract, op1=OP.add)
                    nc.scalar.activation(dAN[:, :], dAN[:, :], AF.Exp)

                    NT = phA.tile([128, 128], BF16, tag="NT", bufs=3, name=f"NT{rj}")
                    nc.vector.tensor_tensor(out=NT[:, :], in0=gp[:, 0:128],
                                            in1=dAN[:, 128:256], op=OP.mult)
                    AT = phA.tile([128, 128], F32R, tag="AT", bufs=6, name=f"AT{rj}")
                    nc.vector.tensor_tensor(out=AT[:, :], in0=gp[:, 128:256],
                                            in1=dAN[:, 0:128], op=OP.mult)
                    AT_t[(hl, j)] = AT

                    ntp = psA.tile([128, 128], BF16, tag="psA", name=f"ntp{rj}")
                    nc.tensor.transpose(ntp[:, :], NT[:, :], identb_s[:, :])
                    Nb = invp.tile([128, 128], BF16, tag="Nb", name=f"Nb{rj}")
                    nc.vector.tensor_copy(Nb[:, :], ntp[:, :])

                    # TmT = (I+NT^64)...(I+NT^2)(I-NT), bf16 doubling
                    Rb = invp.tile([128, 128], BF16, tag="Rb", name=f"Rb{rj}")
                    nc.gpsimd.tensor_tensor(out=Rb[:, :], in0=identb_s[:, :],
                                            in1=NT[:, :], op=OP.subtract)
                    Np, NTp = Nb, NT
                    for p in [2, 4]:
                        last = (p == 4)
                        sq2 = psA.tile([128, 256], F32, tag="psA", name=f"sq{rj}_{p}")
                        nc.tensor.matmul(sq2[:, 0:128], NTp[:, :], Np[:, :],
                                         start=True, stop=True)
                        if not last:
                            nc.tensor.matmul(sq2[:, 128:256], Np[:, :], NTp[:, :],
                                             start=True, stop=True)
                        pair = invp.tile([128, 256], BF16, tag="pair",
                                         name=f"pr{rj}_{p}")
                        if last:
                            nc.vector.tensor_copy(pair[:, 0:128], sq2[:, 0:128])
                        else:
                            nc.vector.tensor_copy(pair[:, :], sq2[:, :])
                        Np, NTp = pair[:, 0:128], pair[:, 128:256]
                        rp = psA.tile([128, 128], F32, tag="psA", name=f"rp{rj}_{p}")
                        nc.tensor.matmul(rp[:, :], Np[:, :], Rb[:, :],
                                         start=True, stop=True)
                        if not last:
                            Rb2 = invp.tile([128, 128], BF16, tag="Rb",
                                            name=f"Rb{rj}_{p}")
                            nc.vector.tensor_tensor(out=Rb2[:, :], in0=Rb[:, :],
                                                    in1=rp[:, :], op=OP.add)
                            Rb = Rb2
                        else:
                            TmT = phA.tile([128, 128], F32R, tag="TmT", bufs=6,
                                           name=f"TmT{rj}")
                            nc.vector.tensor_tensor(out=TmT[:, :], in0=Rb[:, :],
                                                    in1=rp[:, :], op=OP.add)
                            TmT_t[(hl, j)] = TmT

                    vp = psA.tile([128, 256], F32R, tag="psA", name=f"vp{rj}")
                    nc.tensor.transpose(vp[:, 0:128], vT[2 * hl][:, sl],
                                        identf_s[:, :])
                    nc.tensor.transpose(vp[:, 128:256], vT[2 * hl + 1][:, sl],
                                        identf_s[:, :])
                    Vb = phA.tile([128, 256], F32R, tag="Vb", bufs=6, name=f"Vb{rj}")
                    nc.vector.tensor_scalar_mul(Vb[:, :], vp[:, :],
                                                b_cols[:, rj:rj + 1])
                    Vb_t[(hl, j)] = Vb

                    kp = psA.tile([128, 128], F32R, tag="psA", name=f"kp{rj}")
                    nc.tensor.transpose(kp[:, :], kn[:, sl], identf_s[:, :])
                    Kp = phA.tile([128, 128], F32R, tag="Kp", bufs=6, name=f"Kp{rj}")
                    nc.scalar.activation(Kp[:, :], kp[:, :], AF.Copy,
                                         scale=ed_cols[:, rj:rj + 1])
                    Kp_t[(hl, j)] = Kp

            # ---- phase B: sequential state recurrence ----
            for j in range(NCH):
                for hl in range(HL):
                    rj = hl * NCH + j
                    sl = slice(j * 128, (j + 1) * 128)
                    kn, qn = qkT[2 + hl], qkT[hl]
                    TmT, AT, Vb, Kp = (TmT_t[(hl, j)], AT_t[(hl, j)],
                                       Vb_t[(hl, j)], Kp_t[(hl, j)])

                    if j == 0:
                        RHS = Vb
                    else:
                        wr = psB.tile([128, 256], F32, tag="psB", name=f"wr{rj}")
                        nc.tensor.matmul(wr[:, :], kn[:, sl], S_s[hl][:, :],
                                         start=True, stop=True)
                        RHS = phB.tile([128, 256], F32R, tag="RHS", name=f"RHS{rj}")
                        nc.vector.scalar_tensor_tensor(
                            out=RHS[:, :], in0=wr[:, :],
                            scalar=nbl_cols[:, rj:rj + 1], in1=Vb[:, :],
                            op0=OP.mult, op1=OP.add)

                    up = psB.tile([128, 256], F32, tag="psB", name=f"up{rj}")
                    nc.tensor.matmul(up[:, :], TmT[:, :], RHS[:, :],
                                     start=True, stop=True)
                    U = phB.tile([128, 256], F32R, tag="U", name=f"U{rj}")
                    nc.vector.tensor_copy(U[:, :], up[:, :])

                    t2 = psB.tile([128, 256], F32, tag="psB", name=f"t2{rj}")
                    nc.tensor.matmul(t2[:, :], AT[:, :], U[:, :],
                                     start=True, stop=True)
                    o_raw = og_s[:, j, hl * DV:(hl + 1) * DV]
                    if j == 0:
                        nc.vector.tensor_copy(o_raw, t2[:, :])
                    else:
                        t2s = phB.tile([128, 256], F32, tag="t2s", name=f"t2s{rj}")
                        nc.scalar.activation(t2s[:, :], t2[:, :], AF.Copy)
                        t1 = psB.tile([128, 256], F32, tag="psB", name=f"t1{rj}")
                        nc.tensor.matmul(t1[:, :], qn[:, sl], S_s[hl][:, :],
                                         start=True, stop=True)
                        nc.vector.scalar_tensor_tensor(
                            out=o_raw, in0=t1[:, :], scalar=lam_cols[:, rj:rj + 1],
                            in1=t2s[:, :], op0=OP.mult, op1=OP.add)

                    kup = psB.tile([128, 256], F32, tag="psB", name=f"kup{rj}")
                    nc.tensor.matmul(kup[:, :], Kp[:, :], U[:, :],
                                     start=True, stop=True)
                    if j == 0:
                        nc.vector.tensor_copy(S_s[hl][:, :], kup[:, :])
                    else:
                        nc.vector.scalar_tensor_tensor(
                            out=S_s[hl][:, :], in0=S_s[hl][:, :],
                            scalar=gamb_s[:, rj:rj + 1], in1=kup[:, :],
                            op0=OP.mult, op1=OP.add)

                    # rmsnorm stats; sqrt batched per 4-chunk group below
                    osq = phB.tile([128, 256], F32, tag="osq", name=f"osq{rj}")
                    nc.scalar.activation(osq[:, :], o_raw, AF.Square,
                                         accum_out=ssq_all[:, j * HL + hl:j * HL + hl + 1])

            if debug:
                nc.sync.dma_start(out=dbg["qk0"][:, :], in_=qkT[0][:, :].bitcast(F32))
                nc.sync.dma_start(out=dbg["qk2"][:, :], in_=qkT[2][:, :].bitcast(F32))
                nc.sync.dma_start(out=dbg["v0"][:, :], in_=vT[0][:, :].bitcast(F32))
                nc.sync.dma_start(out=dbg["ba"][:, :], in_=ba_s[:, :])
                nc.sync.dma_start(out=dbg["crows"][:, :], in_=c_rows[:, :])
                nc.sync.dma_start(out=dbg["lam"][:, :], in_=lam_rows[:, :])
                nc.sync.dma_start(out=dbg["ed"][:, :], in_=ed_rows[:, :])
                nc.sync.dma_start(out=dbg["bet"][:, :], in_=bet_rows[:, :])
                nc.sync.dma_start(out=dbg["TmT00"][:, :],
                                  in_=TmT_t[(0, 0)][:, :].bitcast(F32))
                nc.sync.dma_start(out=dbg["AT00"][:, :],
                                  in_=AT_t[(0, 0)][:, :].bitcast(F32))
                nc.sync.dma_start(out=dbg["Vb00"][:, :],
                                  in_=Vb_t[(0, 0)][:, :].bitcast(F32))
                nc.sync.dma_start(out=dbg["Kp00"][:, :],
                                  in_=Kp_t[(0, 0)][:, :].bitcast(F32))
                nc.sync.dma_start(out=dbg["S0"][:, :], in_=S_s[0][:, :].bitcast(F32))
                nc.sync.dma_start(
                    out=dbg["og"][:, :],
                    in_=og_s[:, :, :].rearrange("p a b -> p (a b)").bitcast(F32))

            # ---- grouped rmsnorm scale + gate + o_proj ----
            for ct_i in range(4):
                nc.sync.dma_start(
                    out=Wo_s[:, ct_i, :],
                    in_=Wo.rearrange("(ct p) d -> p ct d", p=128)[:, ct_i, :])
            for grp in range(2):
                c0, c1 = grp * 8, grp * 8 + 8
                nc.scalar.activation(rstd_all[:, c0:c1], ssq_all[:, c0:c1],
                                     AF.Sqrt, scale=1.0 / DV, bias=epsn_col[:, :])
                nc.vector.reciprocal(rstd_all[:, c0:c1], rstd_all[:, c0:c1])
                for tt in range(grp * 4, grp * 4 + 4):
                    for hl in range(HL):
                        cc = tt * HL + hl
                        nc.vector.scalar_tensor_tensor(
                            out=og_s[:, tt, hl * DV:(hl + 1) * DV],
                            in0=og_s[:, tt, hl * DV:(hl + 1) * DV],
                            scalar=rstd_all[:, cc:cc + 1],
                            in1=gvsnw[:, tt, hl * DV:(hl + 1) * DV],
                            op0=OP.mult, op1=OP.mult)
            for tt in range(NCH):
                otp = psB.tile([128, 512], F32R, tag="psB", name=f"otp{tt}")
                for cs in range(4):
                    nc.tensor.transpose(otp[:, cs * 128:(cs + 1) * 128],
                                        og_s[:, tt, cs * 128:(cs + 1) * 128],
                                        identf_s[:, :])
                oTt = outp.tile([128, 512], F32R, tag="oT", name=f"oT{tt}")
                nc.vector.tensor_copy(oTt[:, :], otp[:, :])
                ot_out = outp.tile([128, D], F32, tag="oout", name=f"oo{tt}")
                ops2 = [psB.tile([128, 512], F32, tag="psB", name=f"op{tt}_{h}")
                        for h in range(2)]
                for cs in range(4):
                    for dh in range(2):
                        nc.tensor.matmul(ops2[dh][:, :],
                                         oTt[:, cs * 128:(cs + 1) * 128],
                                         Wo_s[:, cs, dh * 512:(dh + 1) * 512],
                                         start=(cs == 0), stop=(cs == 3))
                for dh in range(2):
                    nc.vector.tensor_copy(ot_out[:, dh * 512:(dh + 1) * 512],
                                          ops2[dh][:, :])
                nc.sync.dma_start(out=outD[tt * 128:(tt + 1) * 128, :],
                                  in_=ot_out[:, :])

    nc.compile()
    return nc


def _prep_core_inputs(inputs, core):
    b = core // 4
    hp = (core % 4) * 2
    x = np.asarray(inputs["x"], np.float32)
    Wq = np.asarray(inputs["Wq"], np.float32)
    Wk = np.asarray(inputs["Wk"], np.float32)
    Wv_f = np.asarray(inputs["Wv"], np.float32)
    Wg_f = np.asarray(inputs["Wg"], np.float32)
    Wb = np.asarray(inputs["Wb"], np.float32)
    Wa = np.asarray(inputs["Wa"], np.float32)
    Wo_f = np.asarray(inputs["Wo"], np.float32)
    conv_q = np.asarray(inputs["conv_q"], np.float32)
    conv_k = np.asarray(inputs["conv_k"], np.float32)
    conv_v = np.asarray(inputs["conv_v"], np.float32)
    A_log = np.asarray(inputs["A_log"], np.float32)
    dt_bias = np.asarray(inputs["dt_bias"], np.float32)
    norm_w = np.asarray(inputs["norm_w"], np.float32)

    h0, h1 = hp, hp + 1
    xTc = np.ascontiguousarray(x[b].T)
    Wqk_a = np.concatenate(
        [Wq[:, h0 * DK:(h0 + 1) * DK], Wq[:, h1 * DK:(h1 + 1) * DK],
         Wk[:, h0 * DK:(h0 + 1) * DK], Wk[:, h1 * DK:(h1 + 1) * DK]], axis=1)
    Wv_sh = np.ascontiguousarray(Wv_f[:, h0 * DV:(h0 + 2) * DV])
    Wg_sh = np.ascontiguousarray(Wg_f[:, h0 * DV:(h0 + 2) * DV])
    Wba_a = np.stack([Wb[:, h0], Wb[:, h1], Wa[:, h0], Wa[:, h1]], axis=1)
    Wo_sh = np.ascontiguousarray(Wo_f[h0 * DV:(h0 + 2) * DV, :]
                                 * np.tile(norm_w, 2)[:, None])

    convd_a = np.zeros((8, KC, 128, 128), np.float32)
    cblocks = [conv_q[h0 * DK:(h0 + 1) * DK], conv_q[h1 * DK:(h1 + 1) * DK],
               conv_k[h0 * DK:(h0 + 1) * DK], conv_k[h1 * DK:(h1 + 1) * DK],
               conv_v[h0 * DV:h0 * DV + 128], conv_v[h0 * DV + 128:(h0 + 1) * DV],
               conv_v[h1 * DV:h1 * DV + 128], conv_v[h1 * DV + 128:(h1 + 1) * DV]]
    ii = np.arange(128)
    for blk, w in enumerate(cblocks):
        for tap in range(KC):
            convd_a[blk, tap, ii, ii] = w[:, tap]

    adt_a = np.zeros((16, 2), np.float32)
    for hl in range(HL):
        adt_a[hl * NCH:(hl + 1) * NCH, 0] = -dt_bias[hp + hl]
        adt_a[hl * NCH:(hl + 1) * NCH, 1] = np.exp(A_log[hp + hl])

    tri = np.triu(np.ones((128, 128), bool))          # row i <= col t
    maskI_a = np.where(tri, 0.0, NEG).astype(np.float32)
    maskS_a = np.where(np.triu(np.ones((128, 128), bool), 1), 0.0,
                       NEG).astype(np.float32)
    ident = np.eye(128, dtype=np.float32)

    return {
        "xT": xTc, "Wqk": np.ascontiguousarray(Wqk_a), "Wv": Wv_sh, "Wg": Wg_sh,
        "Wba": np.ascontiguousarray(Wba_a), "Wo": Wo_sh, "convd": convd_a,
        "adt": adt_a, "maskI": maskI_a, "maskS": maskS_a, "identf": ident,
        "identg": np.eye(16, dtype=np.float32),
        "identb": ident.astype(ml_dtypes.bfloat16),
    }


def kernel(**inputs):
    if "nc" not in _cache:
        _cache["nc"] = build_kernel()
    nc = _cache["nc"]
    in_maps = [_prep_core_inputs(inputs, core) for core in range(8)]
    res = run_bass_kernel_spmd(nc, in_maps, core_ids=list(range(8)))
    out = np.zeros((B, T, D), np.float32)
    for b in range(B):
        for g in range(4):
            out[b] += res.results[4 * b + g]["out"]
    return out



# revision 20
# speedup vs baseline: 1.3110x; 1.3110x over previous
"""FLA GatedDeltaNet layer on 8 Trainium2 NeuronCores.

Sharding: data-parallel over batch (2 groups) x tensor-parallel over heads
(4 shards of 2 heads). Each core computes its batch element with its 2 heads
end-to-end (projections, short conv, gated delta rule recurrence, gated
RMSNorm, o_proj partial). Host sums the 4 o_proj partials per batch element.

Recurrence uses the chunked WY form (chunk C=128):
  S_t = exp(g_t) S_{t-1};  u_t = beta_t (v_t - k_t^T S_t);  S_t += k_t u_t^T
  per chunk: (I + N) U = beta (V - Lam K S0),  N[t,i] = b_t e^{c_t-c_i} k_t.k_i
  TmT = transposed (I+N)^{-1} via product-form doubling (N nilpotent),
  O = Lam Q S0 + (QK^T . decay) U,  S1 = gam S0 + ((gam/Lam) K)^T U.

v2 changes vs baseline:
  - weights + x fed as bf16 from host (halves DMA + LDWEIGHTS cost)
  - decay-row broadcast via gpsimd partition_broadcast (was fp32-exact
    dual-pass PE matmuls)
  - activation tables preloaded at t=0 under the initial DMA shadow
  - scalar decay-stream computed inside the projection phase (ba-part after
    the ba block, norm-part after the last q/k block)
  - dAN tiles precomputed (vector/scalar run ahead of the PE phase-A loop)
  - phase A emitted with head-pair interleaving for PE queue density
  - o_proj fused per-chunk into phase B, interleaved to fill S-chain stalls
"""
import sys

if "/opt/trn_rl_repo" not in sys.path:
    sys.path.insert(0, "/opt/trn_rl_repo")

import numpy as np
import ml_dtypes

import concourse.bass as bass
import concourse.bacc as bacc
import concourse.mybir as mybir
import concourse.tile as tile
from concourse.bass_utils import run_bass_kernel_spmd

F32 = mybir.dt.float32
F32R = mybir.dt.float32r
BF16 = mybir.dt.bfloat16
AF = mybir.ActivationFunctionType
OP = mybir.AluOpType

B, T, D = 2, 1024, 1024
H, DK, DV, KC = 8, 128, 256, 4
HL = 2              # heads per core
C = 128             # chunk length
NCH = T // C        # 8 chunks
NEG = -1e30
EPS = 1e-5
QSCALE = float(DK) ** -0.5

_cache = {}


def build_kernel(debug=False):
    nc = bacc.Bacc(None, target_bir_lowering=False)

    xT = nc.dram_tensor("xT", [D, T], BF16, kind="ExternalInput")
    Wqk = nc.dram_tensor("Wqk", [D, 512], BF16, kind="ExternalInput")
    Wv = nc.dram_tensor("Wv", [D, 512], BF16, kind="ExternalInput")
    Wg = nc.dram_tensor("Wg", [D, 512], BF16, kind="ExternalInput")
    Wba = nc.dram_tensor("Wba", [D, 4], BF16, kind="ExternalInput")
    Wo = nc.dram_tensor("Wo", [HL * DV, D], BF16, kind="ExternalInput")
    convd = nc.dram_tensor("convd", [8, KC, 128, 128], BF16, kind="ExternalInput")
    adt = nc.dram_tensor("adt", [16, 2], F32, kind="ExternalInput")
    maskI = nc.dram_tensor("maskI", [128, 128], F32, kind="ExternalInput")
    maskS = nc.dram_tensor("maskS", [128, 128], F32, kind="ExternalInput")
    identf = nc.dram_tensor("identf", [128, 128], F32R, kind="ExternalInput")
    identg = nc.dram_tensor("identg", [16, 16], F32, kind="ExternalInput")
    identb = nc.dram_tensor("identb", [128, 128], BF16, kind="ExternalInput")
    outD = nc.dram_tensor("out", [T, D], BF16, kind="ExternalOutput")

    with tile.TileContext(nc, pool_alloc_mode="queue") as tc, \
         tc.tile_pool(name="res", bufs=1) as res:

        # ---------------- activation table warmup (under DMA shadow) -------
        warm_in = res.tile([1, 2], F32)
        warm_out = res.tile([1, 2], F32)
        nc.vector.memset(warm_in, 0.25)
        for af in [AF.Copy, AF.Silu, AF.Square, AF.Sigmoid, AF.Ln, AF.Exp,
                   AF.Sqrt]:
            nc.scalar.activation(warm_out, warm_in, af)

        # ---------------- resident (small) loads ----------------
        Wo_s = res.tile([128, 4, D], BF16)
        Wba_s = res.tile([128, 8, 4], BF16)
        nc.sync.dma_start(out=Wba_s, in_=Wba.rearrange("(dt p) c -> p dt c", p=128))
        adt_s = res.tile([16, 2], F32)
        nc.sync.dma_start(out=adt_s, in_=adt[:, :])
        maskIS_s = res.tile([128, 256], F32)
        nc.sync.dma_start(out=maskIS_s[:, 0:128], in_=maskI[:, :])
        nc.sync.dma_start(out=maskIS_s[:, 128:256], in_=maskS[:, :])
        identf_s = res.tile([128, 128], F32R)
        nc.sync.dma_start(out=identf_s, in_=identf[:, :])
        identg_s = res.tile([16, 16], F32)
        nc.sync.dma_start(out=identg_s, in_=identg[:, :])
        identb_s = res.tile([128, 128], BF16)
        nc.sync.dma_start(out=identb_s, in_=identb[:, :])

        zerob_col = res.tile([128, 4], BF16)
        nc.vector.memset(zerob_col, 0.0)
        onesb_col = res.tile([128, 1], BF16)
        nc.vector.memset(onesb_col, 1.0)
        eps6_col = res.tile([128, 1], F32)
        nc.vector.memset(eps6_col, 1e-6)
        epsn_col = res.tile([128, 1], F32)
        nc.vector.memset(epsn_col, EPS)
        zeros16 = res.tile([16, 128], F32)
        nc.vector.memset(zeros16, 0.0)

        # persistent per-block activation tiles
        qkT = [res.tile([128, T], F32R, tag=f"qkT{i}", name=f"qkT{i}") for i in range(4)]
        qkB_all = res.tile([128, 4, T], BF16)   # block order: k0 q0 k1 q1
        nsq_s = [res.tile([1, T], F32, tag=f"nsq{i}", name=f"nsq{i}")
                 for i in range(4)]
        vT = [res.tile([128, T], F32R, tag=f"vT{i}", name=f"vT{i}") for i in range(4)]
        ba_s = res.tile([4, T], F32)
        gvsnw = res.tile([128, NCH, 2 * DV], BF16)    # silu(gv) gate
        S_s = [res.tile([128, DV], F32R, tag=f"S{hl}", name=f"S{hl}") for hl in range(HL)]
        ssq_all = res.tile([128, 16], F32)
        rstd_all = res.tile([128, 16], F32)

        # scalar decay-stream tiles (written during projection phase)
        bagB = res.tile([16, 128], F32)
        bagA = res.tile([16, 128], F32)
        bet_rows = res.tile([16, 128], F32)
        sg_rows = res.tile([16, 128], F32)
        g_rows = res.tile([16, 128], F32)
        c_rows = res.tile([16, 128], F32)
        lnb_rows = res.tile([16, 128], F32)
        cb_rows = res.tile([16, 128], F32)
        lam_rows = res.tile([16, 128], F32)
        lnq4 = [res.tile([1, T], F32, tag=f"lnq{i}", name=f"lnq{i}")
                for i in range(4)]
        lnm_rows = res.tile([16, 128], F32)
        lnn_rows = res.tile([16, 128], F32)
        cqs_rows = res.tile([16, 128], F32)
        cA_rows = res.tile([16, 128], F32)
        cN_rows = res.tile([16, 128], F32)
        ccn_rows = res.tile([16, 128], F32)
        cln_rows = res.tile([16, 128], F32)
        lamq_rows = res.tile([16, 128], F32)
        edn_rows = res.tile([16, 128], F32)
        lamn_rows = res.tile([16, 128], F32)
        nbl_rows = res.tile([16, 128], F32)
        glast_row = res.tile([1, 16], F32)
        gamb_s = res.tile([128, 16], F32)
        ccb_flat = res.tile([1, 2, 16 * 128], F32)
        bc_all = res.tile([128, 16, 256], F32)   # gpsimd-broadcast decay rows
        cols_t = {nm: res.tile([128, 16], F32, tag=f"cols_{nm}", name=f"cols_{nm}")
                  for nm in ["ccn", "lam", "ed", "b", "nbl"]}

        # ======== projection scope (xT/Wg freed afterwards) ========
        with tc.tile_pool(name="xp", bufs=1) as xp, \
             tc.tile_pool(name="wstream", bufs=3) as wstream, \
             tc.tile_pool(name="cstream", bufs=2) as cstream, \
             tc.tile_pool(name="pre", bufs=2) as pre_pool, \
             tc.tile_pool(name="psJ", bufs=4, space="PSUM") as psJ, \
             tc.tile_pool(name="psn", bufs=2, space="PSUM") as psn:

            xT_s = xp.tile([128, 8, T], BF16)          # [p, dtile, t]
            for dt_i in range(8):
                nc.sync.dma_start(
                    out=xT_s[:, dt_i, :],
                    in_=xT.rearrange("(dt p) t -> p dt t", p=128)[:, dt_i, :])
            Wg_s = xp.tile([128, 8, 512], BF16)

            def do_block(blk):
                nparts = 128 if blk < 8 else 4
                psums = [psJ.tile([nparts, 512], F32, tag="psJ", name=f"pj{blk}_{h}")
                         for h in range(2)]
                if blk < 8:
                    wsrc = Wqk if blk < 4 else Wv
                    cb = blk if blk < 4 else blk - 4
                    wt8 = wstream.tile([128, 8, 128], BF16, tag="w", name=f"w{blk}")
                    nc.sync.dma_start(
                        out=wt8,
                        in_=wsrc.rearrange("(dt p) c -> p dt c", p=128)[:, :, cb * 128:(cb + 1) * 128])
                for d in range(8):
                    wt = wt8[:, d, :] if blk < 8 else Wba_s[:, d, :]
                    for half in range(2):
                        nc.tensor.matmul(
                            psums[half][:, :], wt,
                            xT_s[:, d, half * 512:(half + 1) * 512],
                            start=(d == 0), stop=(d == 7))
                if blk < 8:
                    pret = pre_pool.tile([128, T + 3], BF16, tag="pre", name=f"pre{blk}")
                    nc.vector.tensor_copy(pret[:, 0:3], zerob_col[:, 0:3])
                    nc.vector.tensor_copy(pret[:, 3:3 + 512], psums[0][:, :])
                    nc.scalar.activation(pret[:, 3 + 512:3 + 1024], psums[1][:, :],
                                         AF.Copy)
                    cdt = cstream.tile([128, KC, 128], BF16, tag="cd", name=f"cd{blk}")
                    nc.sync.dma_start(
                        out=cdt, in_=convd.rearrange("b k p m -> p b k m")[:, blk, :, :])
                    dest = vT[blk - 4] if blk >= 4 else qkT[blk]
                    cps2 = [psJ.tile([128, 512], F32, tag="psJ", name=f"cv{blk}_{h}")
                            for h in range(2)]
                    for tap in range(KC):
                        for half in range(2):
                            nc.tensor.matmul(
                                cps2[half][:, :], cdt[:, tap, :],
                                pret[:, half * 512 + tap:half * 512 + tap + 512],
                                start=(tap == 0), stop=(tap == KC - 1))
                    for half in range(2):
                        nc.scalar.activation(dest[:, half * 512:(half + 1) * 512],
                                             cps2[half][:, :], AF.Silu)
                    if blk < 4:
                        # l2 norm stats only; normalization is folded into the
                        # decay scalars (exp-space) below
                        sq = pre_pool.tile([128, T], BF16, tag="sq", name=f"sq{blk}")
                        nc.scalar.activation(sq[:, :], dest[:, :], AF.Square)
                        for q4 in range(4):
                            nsp = psn.tile([1, 256], F32, tag="psn", name=f"ns{blk}_{q4}")
                            nc.tensor.matmul(nsp[:, :], onesb_col[:, :],
                                             sq[:, q4 * 256:(q4 + 1) * 256],
                                             start=True, stop=True)
                            nc.vector.tensor_copy(
                                nsq_s[blk][:, q4 * 256:(q4 + 1) * 256], nsp[:, :])
                        bmap = {0: 1, 1: 3, 2: 0, 3: 2}
                        nc.gpsimd.tensor_copy(qkB_all[:, bmap[blk], :], dest[:, :])
                else:
                    for half in range(2):
                        nc.vector.tensor_copy(ba_s[:, half * 512:(half + 1) * 512],
                                              psums[half][:, :])

            # ---- ba block first, then scalar stream part 1 ----
            do_block(8)

            for f in range(2):
                nc.sync.dma_start(
                    out=bagB[f * NCH:(f + 1) * NCH, :],
                    in_=ba_s[f:f + 1, :].rearrange("o (j t) -> o j t", j=NCH))
                nc.sync.dma_start(
                    out=bagA[f * NCH:(f + 1) * NCH, :],
                    in_=ba_s[2 + f:3 + f, :].rearrange("o (j t) -> o j t", j=NCH))
            nc.scalar.activation(bet_rows[:, :], bagB[:, :], AF.Sigmoid)
            nc.scalar.activation(sg_rows[:, :], bagA[:, :], AF.Sigmoid,
                                 scale=-1.0, bias=adt_s[:, 0:1])
            # g = exp(A_log) * ln(sigmoid(-(a+dt_bias)))  [= -expA * softplus]
            nc.scalar.activation(g_rows[:, :], sg_rows[:, :], AF.Ln)
            nc.scalar.activation(g_rows[:, :], g_rows[:, :], AF.Copy,
                                 scale=adt_s[:, 1:2])
            nc.vector.tensor_tensor_scan(c_rows[:, :], g_rows[:, :], zeros16[:, :],
                                         0.0, op0=OP.add, op1=OP.add)
            nc.scalar.activation(lnb_rows[:, :], bet_rows[:, :], AF.Ln)
            nc.vector.tensor_tensor(out=cb_rows[:, :], in0=c_rows[:, :],
                                    in1=lnb_rows[:, :], op=OP.add)
            nc.scalar.activation(lam_rows[:, :], c_rows[:, :], AF.Exp)

            # b_cols + gamb (ba-dependent only)
            bps = psn.tile([128, 16], F32, tag="psn", name="tc_b")
            nc.tensor.transpose(bps[:, :], bet_rows[:, :], identg_s[:, :])
            nc.vector.tensor_copy(cols_t["b"][:, :], bps[:, :])
            glast_ps = psn.tile([1, 16], F32, tag="psn", name="glast_ps")
            nc.tensor.transpose(glast_ps[:, :], lam_rows[:, 127:128],
                                identg_s[:, :])
            nc.vector.tensor_copy(glast_row[:, :], glast_ps[:, :])
            nc.gpsimd.partition_broadcast(gamb_s[:, :], glast_row[:, :])

            # ---- q/k blocks ----
            for blk in [2, 0, 3, 1]:
                do_block(blk)

            # ---- scalar stream part 2 (needs all q/k norm stats) ----
            for i in range(4):
                nc.scalar.activation(lnq4[i][:, :], nsq_s[i][:, :], AF.Ln,
                                     bias=eps6_col[0:1, :])
            for f in range(2):
                nc.sync.dma_start(
                    out=lnm_rows[f * NCH:(f + 1) * NCH, :],
                    in_=lnq4[f][0:1, :].rearrange("o (j t) -> o j t", j=NCH))
                nc.sync.dma_start(
                    out=lnn_rows[f * NCH:(f + 1) * NCH, :],
                    in_=lnq4[2 + f][0:1, :].rearrange("o (j t) -> o j t", j=NCH))

            nc.vector.tensor_scalar_add(cqs_rows[:, :], c_rows[:, :],
                                        float(np.log(QSCALE)))
            nc.vector.scalar_tensor_tensor(
                out=cA_rows[:, :], in0=lnm_rows[:, :], scalar=-0.5,
                in1=cqs_rows[:, :], op0=OP.mult, op1=OP.add)
            nc.vector.scalar_tensor_tensor(
                out=cN_rows[:, :], in0=lnn_rows[:, :], scalar=-0.5,
                in1=cb_rows[:, :], op0=OP.mult, op1=OP.add)
            nc.vector.scalar_tensor_tensor(
                out=ccn_rows[:, :], in0=lnn_rows[:, :], scalar=0.5,
                in1=c_rows[:, :], op0=OP.mult, op1=OP.add)
            nc.vector.scalar_tensor_tensor(
                out=cln_rows[:, :], in0=lnn_rows[:, :], scalar=-0.5,
                in1=c_rows[:, :], op0=OP.mult, op1=OP.add)

            nc.scalar.activation(lamq_rows[:, :], cA_rows[:, :], AF.Exp)
            nc.scalar.activation(edn_rows[:, :], ccn_rows[:, :], AF.Exp,
                                 scale=-1.0, bias=c_rows[:, 127:128])
            nc.scalar.activation(lamn_rows[:, :], cln_rows[:, :], AF.Exp)
            nc.vector.tensor_tensor(out=nbl_rows[:, :], in0=bet_rows[:, :],
                                    in1=lamn_rows[:, :], op=OP.mult)
            nc.scalar.activation(nbl_rows[:, :], nbl_rows[:, :], AF.Copy,
                                 scale=-1.0)

            nc.sync.dma_start(
                out=ccb_flat[0:1, 0, :].rearrange("p (r t) -> p r t", r=16),
                in_=cA_rows[:, :])
            nc.sync.dma_start(
                out=ccb_flat[0:1, 1, :].rearrange("p (r t) -> p r t", r=16),
                in_=cN_rows[:, :])

            for nm, rt in [("ccn", ccn_rows), ("lam", lamq_rows),
                           ("ed", edn_rows), ("nbl", nbl_rows)]:
                ps = psn.tile([128, 16], F32, tag="psn", name=f"tc_{nm}")
                nc.tensor.transpose(ps[:, :], rt[:, :], identg_s[:, :])
                nc.vector.tensor_copy(cols_t[nm][:, :], ps[:, :])

            # decay-row broadcasts (gpsimd; overlaps v/gv projections)
            for rj in range(16):
                nc.gpsimd.partition_broadcast(
                    bc_all[:, rj, :], ccb_flat[0:1, :, rj * 128:(rj + 1) * 128])

            # ---- v blocks ----
            for blk in [4, 5, 6, 7]:
                do_block(blk)

            # ---- gv projection (row-major) + silu ----
            for dt_i in range(8):
                nc.sync.dma_start(
                    out=Wg_s[:, dt_i, :],
                    in_=Wg.rearrange("(dt p) c -> p dt c", p=128)[:, dt_i, :])
            for tt in range(NCH):
                gps = psJ.tile([128, 512], F32, tag="psJ", name=f"gv{tt}")
                for d in range(8):
                    nc.tensor.matmul(gps[:, :], xT_s[:, d, tt * 128:(tt + 1) * 128],
                                     Wg_s[:, d, :], start=(d == 0), stop=(d == 7))
                nc.scalar.activation(gvsnw[:, tt, :], gps[:, :], AF.Silu)

            # Wo stream-in at the end of the projection phase
            for ct_i in range(4):
                nc.sync.dma_start(
                    out=Wo_s[:, ct_i, :],
                    in_=Wo.rearrange("(ct p) d -> p ct d", p=128)[:, ct_i, :])

        ccn_cols, lam_cols, ed_cols, nbl_cols = (
            cols_t["ccn"], cols_t["lam"], cols_t["ed"], cols_t["nbl"])
        b_cols = cols_t["b"]

        # ======== recurrence scope (reuses xT/Wg space) ========
        with tc.tile_pool(name="phA", bufs=2) as phA, \
             tc.tile_pool(name="invp", bufs=6) as invp, \
             tc.tile_pool(name="phB", bufs=3) as phB, \
             tc.tile_pool(name="outp", bufs=2) as outp, \
             tc.tile_pool(name="psA", bufs=3, space="PSUM") as psA, \
             tc.tile_pool(name="psB", bufs=3, space="PSUM") as psB, \
             tc.tile_pool(name="psO", bufs=2, space="PSUM") as psO:

            # ---- phase A0: dAN precompute (vector+scalar run ahead) ----
            dAN_t = {}
            for j in range(NCH):
                for hl in range(HL):
                    rj = hl * NCH + j
                    dANf = phA.tile([128, 256], F32, tag="dANf", bufs=3,
                                    name=f"dANf{rj}")
                    nc.vector.scalar_tensor_tensor(
                        out=dANf[:, :], in0=bc_all[:, rj, :],
                        scalar=ccn_cols[:, rj:rj + 1],
                        in1=maskIS_s[:, :], op0=OP.subtract, op1=OP.add)
                    dAN = phA.tile([128, 256], BF16, tag="dAN", bufs=17,
                                   name=f"dAN{rj}")
                    nc.scalar.activation(dAN[:, :], dANf[:, :], AF.Exp)
                    dAN_t[(hl, j)] = dAN

            # ---- phase A: chunk-parallel precompute, head-pair interleaved --
            TmT_t, AT_t, Vb_t, Kp_t = {}, {}, {}, {}
            for j in range(NCH):
                sl = slice(j * 128, (j + 1) * 128)
                gp2, NT2, AT2, ntp2, Nb2, Rb2, vp2 = {}, {}, {}, {}, {}, {}, {}
                for hl in range(HL):
                    rj = hl * NCH + j
                    kb = qkB_all[:, 2 * hl, :]
                    gp = psA.tile([128, 256], F32, tag="psA", name=f"gp{rj}")
                    nc.tensor.matmul(gp[:, :].rearrange("p (b t) -> p b t", b=2),
                                     kb[:, sl], qkB_all[:, 2 * hl:2 * hl + 2, sl],
                                     start=True, stop=True)
                    gp2[hl] = gp
                for hl in range(HL):
                    rj = hl * NCH + j
                    dAN = dAN_t[(hl, j)]
                    NT = phA.tile([128, 128], BF16, tag="NT", bufs=3,
                                  name=f"NT{rj}")
                    nc.vector.tensor_tensor(out=NT[:, :], in0=gp2[hl][:, 0:128],
                                            in1=dAN[:, 128:256], op=OP.mult)
                    NT2[hl] = NT
                    AT = phA.tile([128, 128], F32R, tag="AT", bufs=6,
                                  name=f"AT{rj}")
                    nc.vector.tensor_tensor(out=AT[:, :], in0=gp2[hl][:, 128:256],
                                            in1=dAN[:, 0:128], op=OP.mult)
                    AT_t[(hl, j)] = AT
                for hl in range(HL):
                    rj = hl * NCH + j
                    ntp = psA.tile([128, 128], BF16, tag="psA", name=f"ntp{rj}")
                    nc.tensor.transpose(ntp[:, :], NT2[hl][:, :], identb_s[:, :])
                    ntp2[hl] = ntp
                for hl in range(HL):
                    rj = hl * NCH + j
                    Nb = invp.tile([128, 128], BF16, tag="Nb", name=f"Nb{rj}")
                    nc.vector.tensor_copy(Nb[:, :], ntp2[hl][:, :])
                    Nb2[hl] = Nb
                    Rb = invp.tile([128, 128], BF16, tag="Rb", name=f"Rb{rj}")
                    nc.gpsimd.tensor_tensor(out=Rb[:, :], in0=identb_s[:, :],
                                            in1=NT2[hl][:, :], op=OP.subtract)
                    Rb2[hl] = Rb
                # TmT = (I+NT^4)(I+NT^2)(I-NT), bf16 doubling, pair-interleaved
                Np2, NTp2 = dict(Nb2), dict(NT2)
                for p in [2, 4]:
                    last = (p == 4)
                    sqs, rps = {}, {}
                    for hl in range(HL):
                        rj = hl * NCH + j
                        sq2 = psA.tile([128, 256], F32, tag="psA",
                                       name=f"sq{rj}_{p}")
                        nc.tensor.matmul(sq2[:, 0:128], NTp2[hl][:, :],
                                         Np2[hl][:, :], start=True, stop=True)
                        if not last:
                            nc.tensor.matmul(sq2[:, 128:256], Np2[hl][:, :],
                                             NTp2[hl][:, :], start=True, stop=True)
                        sqs[hl] = sq2
                    for hl in range(HL):
                        rj = hl * NCH + j
                        pair = invp.tile([128, 256], BF16, tag="pair",
                                         name=f"pr{rj}_{p}")
                        if last:
                            nc.vector.tensor_copy(pair[:, 0:128], sqs[hl][:, 0:128])
                        else:
                            nc.vector.tensor_copy(pair[:, :], sqs[hl][:, :])
                        Np2[hl], NTp2[hl] = pair[:, 0:128], pair[:, 128:256]
                    for hl in range(HL):
                        rj = hl * NCH + j
                        rp = psA.tile([128, 128], F32, tag="psA",
                                      name=f"rp{rj}_{p}")
                        nc.tensor.matmul(rp[:, :], Np2[hl][:, :], Rb2[hl][:, :],
                                         start=True, stop=True)
                        rps[hl] = rp
                    for hl in range(HL):
                        rj = hl * NCH + j
                        if not last:
                            Rbn = invp.tile([128, 128], BF16, tag="Rb",
                                            name=f"Rb{rj}_{p}")
                            nc.vector.tensor_tensor(out=Rbn[:, :], in0=Rb2[hl][:, :],
                                                    in1=rps[hl][:, :], op=OP.add)
                            Rb2[hl] = Rbn
                        else:
                            TmT = phA.tile([128, 128], F32R, tag="TmT", bufs=6,
                                           name=f"TmT{rj}")
                            nc.vector.tensor_tensor(out=TmT[:, :], in0=Rb2[hl][:, :],
                                                    in1=rps[hl][:, :], op=OP.add)
                            TmT_t[(hl, j)] = TmT
                for hl in range(HL):
                    rj = hl * NCH + j
                    vp = psA.tile([128, 256], F32R, tag="psA", name=f"vp{rj}")
                    nc.tensor.transpose(vp[:, 0:128], vT[2 * hl][:, sl],
                                        identf_s[:, :])
                    nc.tensor.transpose(vp[:, 128:256], vT[2 * hl + 1][:, sl],
                                        identf_s[:, :])
                    vp2[hl] = vp
                for hl in range(HL):
                    rj = hl * NCH + j
                    Vb = phA.tile([128, 256], F32R, tag="Vb", bufs=6,
                                  name=f"Vb{rj}")
                    nc.vector.tensor_scalar_mul(Vb[:, :], vp2[hl][:, :],
                                                b_cols[:, rj:rj + 1])
                    Vb_t[(hl, j)] = Vb
                kps = {}
                for hl in range(HL):
                    rj = hl * NCH + j
                    kp = psA.tile([128, 128], F32R, tag="psA", name=f"kp{rj}")
                    nc.tensor.transpose(kp[:, :], qkT[2 + hl][:, sl],
                                        identf_s[:, :])
                    kps[hl] = kp
                for hl in range(HL):
                    rj = hl * NCH + j
                    Kp = phA.tile([128, 128], F32R, tag="Kp", bufs=6,
                                  name=f"Kp{rj}")
                    nc.scalar.activation(Kp[:, :], kps[hl][:, :], AF.Copy,
                                         scale=ed_cols[:, rj:rj + 1])
                    Kp_t[(hl, j)] = Kp

            # ---- phase B: sequential state recurrence + fused o_proj ----
            # o-work for chunk j-1 is interleaved into chunk j's S-chain
            # stalls (in-order engine queues: filler must be emitted between
            # dependent ops).
            owork = {"pend": None}

            def emit_o_transposes(j):
                otp = psO.tile([128, 512], BF16, tag="psO", name=f"otp{j}")
                og = owork["og"]
                for cs in range(4):
                    nc.tensor.transpose(otp[:, cs * 128:(cs + 1) * 128],
                                        og[:, cs * 128:(cs + 1) * 128],
                                        identb_s[:, :])
                owork["otp"] = otp

            def emit_o_copy(j):
                oTt = outp.tile([128, 512], BF16, tag="oT", name=f"oT{j}")
                nc.scalar.activation(oTt[:, :], owork["otp"][:, :], AF.Copy)
                owork["oTt"] = oTt
                owork["oout"] = outp.tile([128, D], BF16, tag="oout",
                                          name=f"oo{j}")

            def emit_o_matmuls(j, dh):
                oTt = owork["oTt"]
                ops = psO.tile([128, 512], F32, tag="psO", name=f"op{j}_{dh}")
                for cs in range(4):
                    nc.tensor.matmul(ops[:, :],
                                     oTt[:, cs * 128:(cs + 1) * 128],
                                     Wo_s[:, cs, dh * 512:(dh + 1) * 512],
                                     start=(cs == 0), stop=(cs == 3))
                nc.scalar.activation(
                    owork["oout"][:, dh * 512:(dh + 1) * 512], ops[:, :],
                    AF.Copy)

            def emit_o_out(j):
                nc.sync.dma_start(out=outD[j * 128:(j + 1) * 128, :],
                                  in_=owork["oout"][:, :])

            def emit_o_gate(j):
                # rmsnorm scale + silu-gate for chunk j -> og tile
                og = outp.tile([128, 512], BF16, tag="og", bufs=3, name=f"og{j}")
                c0 = j * HL
                nc.scalar.activation(rstd_all[:, c0:c0 + 2], ssq_all[:, c0:c0 + 2],
                                     AF.Sqrt, scale=1.0 / DV, bias=epsn_col[:, :])
                nc.vector.reciprocal(rstd_all[:, c0:c0 + 2], rstd_all[:, c0:c0 + 2])
                for hl in range(HL):
                    cc = c0 + hl
                    nc.vector.scalar_tensor_tensor(
                        out=og[:, hl * DV:(hl + 1) * DV],
                        in0=owork["oraw"][hl],
                        scalar=rstd_all[:, cc:cc + 1],
                        in1=gvsnw[:, j, hl * DV:(hl + 1) * DV],
                        op0=OP.mult, op1=OP.mult)
                owork["og"] = og

            for j in range(NCH):
                kn2 = {hl: qkT[2 + hl] for hl in range(HL)}
                qn2 = {hl: qkT[hl] for hl in range(HL)}
                sl = slice(j * 128, (j + 1) * 128)
                pj = owork["pend"]

                # -- S-chain head: wr (PE) then RHS (vector) --
                wrs = {}
                if j > 0:
                    for hl in range(HL):
                        rj = hl * NCH + j
                        wr = psB.tile([128, 256], F32, tag="psB", name=f"wr{rj}")
                        nc.tensor.matmul(wr[:, :], kn2[hl][:, sl], S_s[hl][:, :],
                                         start=True, stop=True)
                        wrs[hl] = wr
                RHS2 = {}
                for hl in range(HL):
                    rj = hl * NCH + j
                    if j == 0:
                        RHS2[hl] = Vb_t[(hl, j)]
                    else:
                        RHS = phB.tile([128, 256], F32R, tag="RHS",
                                       name=f"RHS{rj}")
                        nc.vector.scalar_tensor_tensor(
                            out=RHS[:, :], in0=wrs[hl][:, :],
                            scalar=nbl_cols[:, rj:rj + 1], in1=Vb_t[(hl, j)][:, :],
                            op0=OP.mult, op1=OP.add)
                        RHS2[hl] = RHS

                # PE filler while vector computes RHS: o-transposes of j-1
                if pj is not None:
                    emit_o_transposes(pj)

                ups = {}
                for hl in range(HL):
                    rj = hl * NCH + j
                    up = psB.tile([128, 256], F32, tag="psB", name=f"up{rj}")
                    nc.tensor.matmul(up[:, :], TmT_t[(hl, j)][:, :], RHS2[hl][:, :],
                                     start=True, stop=True)
                    ups[hl] = up
                U2 = {}
                for hl in range(HL):
                    rj = hl * NCH + j
                    U = phB.tile([128, 256], F32R, tag="U", name=f"U{rj}")
                    nc.scalar.activation(U[:, :], ups[hl][:, :], AF.Copy)
                    U2[hl] = U

                # PE filler while scalar copies U: o_proj matmuls of j-1 (1st half)
                if pj is not None:
                    emit_o_copy(pj)
                    emit_o_matmuls(pj, 0)

                t2s_ps, t1s, kups = {}, {}, {}
                for hl in range(HL):
                    rj = hl * NCH + j
                    t2 = psB.tile([128, 256], F32, tag="psB", name=f"t2{rj}")
                    nc.tensor.matmul(t2[:, :], AT_t[(hl, j)][:, :], U2[hl][:, :],
                                     start=True, stop=True)
                    t2s_ps[hl] = t2
                    if j > 0:
                        t1 = psB.tile([128, 256], F32, tag="psB", name=f"t1{rj}")
                        nc.tensor.matmul(t1[:, :], qn2[hl][:, sl], S_s[hl][:, :],
                                         start=True, stop=True)
                        t1s[hl] = t1
                    kup = psB.tile([128, 256], F32, tag="psB", name=f"kup{rj}")
                    nc.tensor.matmul(kup[:, :], Kp_t[(hl, j)][:, :], U2[hl][:, :],
                                     start=True, stop=True)
                    kups[hl] = kup

                # PE filler: o_proj matmuls of j-1 (2nd half)
                if pj is not None:
                    emit_o_matmuls(pj, 1)

                # S update (vector) + o_raw assembly
                for hl in range(HL):
                    rj = hl * NCH + j
                    if j == 0:
                        nc.vector.tensor_copy(S_s[hl][:, :], kups[hl][:, :])
                    else:
                        nc.vector.scalar_tensor_tensor(
                            out=S_s[hl][:, :], in0=S_s[hl][:, :],
                            scalar=gamb_s[:, rj:rj + 1], in1=kups[hl][:, :],
                            op0=OP.mult, op1=OP.add)

                oraw = {}
                for hl in range(HL):
                    rj = hl * NCH + j
                    o_t = phB.tile([128, 256], F32, tag="oraw", bufs=3,
                                   name=f"oraw{rj}")
                    if j == 0:
                        nc.vector.tensor_copy(o_t[:, :], t2s_ps[hl][:, :])
                    else:
                        t2s = phB.tile([128, 256], F32, tag="t2s", name=f"t2s{rj}")
                        nc.scalar.activation(t2s[:, :], t2s_ps[hl][:, :], AF.Copy)
                        nc.vector.scalar_tensor_tensor(
                            out=o_t[:, :], in0=t1s[hl][:, :],
                            scalar=lam_cols[:, rj:rj + 1],
                            in1=t2s[:, :], op0=OP.mult, op1=OP.add)
                    oraw[hl] = o_t
                    osq = phB.tile([128, 256], F32, tag="osq", name=f"osq{rj}")
                    nc.scalar.activation(
                        osq[:, :], o_t[:, :], AF.Square,
                        accum_out=ssq_all[:, j * HL + hl:j * HL + hl + 1])

                if pj is not None:
                    emit_o_out(pj)

                owork["oraw"] = oraw
                emit_o_gate(j)
                owork["pend"] = j

            # tail: o-work for the last chunk
            pj = owork["pend"]
            emit_o_transposes(pj)
            emit_o_copy(pj)
            emit_o_matmuls(pj, 0)
            emit_o_matmuls(pj, 1)
            emit_o_out(pj)

    nc.compile()
    return nc


def _prep_core_inputs(inputs, core):
    b = core // 4
    hp = (core % 4) * 2
    bf = ml_dtypes.bfloat16
    x = np.asarray(inputs["x"], np.float32)
    Wq = np.asarray(inputs["Wq"], np.float32)
    Wk = np.asarray(inputs["Wk"], np.float32)
    Wv_f = np.asarray(inputs["Wv"], np.float32)
    Wg_f = np.asarray(inputs["Wg"], np.float32)
    Wb = np.asarray(inputs["Wb"], np.float32)
    Wa = np.asarray(inputs["Wa"], np.float32)
    Wo_f = np.asarray(inputs["Wo"], np.float32)
    conv_q = np.asarray(inputs["conv_q"], np.float32)
    conv_k = np.asarray(inputs["conv_k"], np.float32)
    conv_v = np.asarray(inputs["conv_v"], np.float32)
    A_log = np.asarray(inputs["A_log"], np.float32)
    dt_bias = np.asarray(inputs["dt_bias"], np.float32)
    norm_w = np.asarray(inputs["norm_w"], np.float32)

    h0, h1 = hp, hp + 1
    xTc = np.ascontiguousarray(x[b].T).astype(bf)
    Wqk_a = np.concatenate(
        [Wq[:, h0 * DK:(h0 + 1) * DK], Wq[:, h1 * DK:(h1 + 1) * DK],
         Wk[:, h0 * DK:(h0 + 1) * DK], Wk[:, h1 * DK:(h1 + 1) * DK]], axis=1)
    Wv_sh = np.ascontiguousarray(Wv_f[:, h0 * DV:(h0 + 2) * DV])
    Wg_sh = np.ascontiguousarray(Wg_f[:, h0 * DV:(h0 + 2) * DV])
    Wba_a = np.stack([Wb[:, h0], Wb[:, h1], Wa[:, h0], Wa[:, h1]], axis=1)
    Wo_sh = np.ascontiguousarray(Wo_f[h0 * DV:(h0 + 2) * DV, :]
                                 * np.tile(norm_w, 2)[:, None])

    convd_a = np.zeros((8, KC, 128, 128), np.float32)
    cblocks = [conv_q[h0 * DK:(h0 + 1) * DK], conv_q[h1 * DK:(h1 + 1) * DK],
               conv_k[h0 * DK:(h0 + 1) * DK], conv_k[h1 * DK:(h1 + 1) * DK],
               conv_v[h0 * DV:h0 * DV + 128], conv_v[h0 * DV + 128:(h0 + 1) * DV],
               conv_v[h1 * DV:h1 * DV + 128], conv_v[h1 * DV + 128:(h1 + 1) * DV]]
    ii = np.arange(128)
    for blk, w in enumerate(cblocks):
        for tap in range(KC):
            convd_a[blk, tap, ii, ii] = w[:, tap]

    adt_a = np.zeros((16, 2), np.float32)
    for hl in range(HL):
        adt_a[hl * NCH:(hl + 1) * NCH, 0] = -dt_bias[hp + hl]
        adt_a[hl * NCH:(hl + 1) * NCH, 1] = np.exp(A_log[hp + hl])

    tri = np.triu(np.ones((128, 128), bool))          # row i <= col t
    maskI_a = np.where(tri, 0.0, NEG).astype(np.float32)
    maskS_a = np.where(np.triu(np.ones((128, 128), bool), 1), 0.0,
                       NEG).astype(np.float32)
    ident = np.eye(128, dtype=np.float32)

    return {
        "xT": xTc, "Wqk": np.ascontiguousarray(Wqk_a).astype(bf),
        "Wv": Wv_sh.astype(bf), "Wg": Wg_sh.astype(bf),
        "Wba": np.ascontiguousarray(Wba_a).astype(bf),
        "Wo": Wo_sh.astype(bf), "convd": convd_a.astype(bf),
        "adt": adt_a, "maskI": maskI_a, "maskS": maskS_a, "identf": ident,
        "identg": np.eye(16, dtype=np.float32),
        "identb": ident.astype(bf),
    }


def kernel(**inputs):
    if "nc" not in _cache:
        _cache["nc"] = build_kernel()
    nc = _cache["nc"]
    in_maps = [_prep_core_inputs(inputs, core) for core in range(8)]
    res = run_bass_kernel_spmd(nc, in_maps, core_ids=list(range(8)))
    out = np.zeros((B, T, D), np.float32)
    for b in range(B):
        for g in range(4):
            out[b] += res.results[4 * b + g]["out"].astype(np.float32)
    return out
